# revision 1
# baseline (speedup 1.0000x reference)
"""Trainium2 Bass kernel for the KGTM-style GRU message-passing GNN.

Reference math (per time step, T=3):
    agg_in  = A_in  @ nodes          (per batch)
    agg_out = A_in.T @ nodes
    zv = sigmoid(agg_in@W3wa.T + agg_out@W3wb.T + fn@W3u.T)
    rv = sigmoid(agg_in@W4wa.T + agg_out@W4wb.T + fn@W4u.T)
    hv = tanh   (agg_in@W5wa.T + agg_out@W5wb.T + (rv*fn)@W5u.T)
    fn' = fn + zv*(hv - fn)
    out_t = fn'@Wouta.T + x@Woutb.T + b_out

Mapping: pure data parallel over batch (8 cores x 256 batches, padded to 258
= 43 tiles of 6).  On-chip layout "L2" puts (batch-local, channel) on the
128-partition axis (6*20 = 120 partitions) and the node index n (512) on the
free axis.  Aggregation consumes nodes in layout "L1" [m, (b,h)] as the
matmul stationary operand so its output lands directly in L2:
    agg_L2[(b,h), n] = sum_m nodes_L1[m, (b,h)] * A~[m, n].
GRU gate matmuls use block-diagonal weights kron(I6, W.T) [120,120].  A PE
transpose converts fn' back to L1 for the next step's aggregation.  All
matmuls run as float32r (1 row/cycle, ~1.5e-4 rel err).
"""

import numpy as np

import concourse.bacc as bacc
import concourse.tile as tile
import concourse.mybir as mybir
from concourse.bass_utils import run_bass_kernel_spmd

F32 = mybir.dt.float32
F32R = mybir.dt.float32r

B, N, H, T = 2048, 512, 20, 3
NCORES = 8
BS = B // NCORES          # 256 batches per core
BPER = 6                  # batches per partition tile
TP = BPER * H             # 120 partitions per tile
NT = 43                   # tiles per core (43*6 = 258, 2 batches of zero pad)
BPAD = NT * BPER          # 258
MK = N // 128             # 4 contraction chunks of 128 along m

LAST_RESULTS = None       # stash of the most recent BassKernelResults


def _r(ap):
    return ap.bitcast(F32R)


def build_nc():
    nc = bacc.Bacc("TRN2", target_bir_lowering=False, debug=False,
                   num_devices=NCORES)

    xl1_d = nc.dram_tensor("xl1", [NT, 128, MK, TP], F32, kind="ExternalInput")
    xl2_d = nc.dram_tensor("xl2", [NT, TP, N], F32, kind="ExternalInput")
    ain_t_d = nc.dram_tensor("ain_t", [N, N], F32, kind="ExternalInput")
    ain_d = nc.dram_tensor("ain", [N, N], F32, kind="ExternalInput")
    wnames = ["wz_in", "wz_out", "wz_fn", "wr_in", "wr_out", "wr_fn",
              "wh_in", "wh_out", "wh_fn", "wo_fn", "wo_x"]
    w_d = {w: nc.dram_tensor(w, [TP, TP], F32, kind="ExternalInput")
           for w in wnames}
    bias_d = nc.dram_tensor("bias", [TP, 1], F32, kind="ExternalInput")
    ident_d = nc.dram_tensor("ident", [128, 128], F32, kind="ExternalInput")
    out_d = nc.dram_tensor("out", [T, NT, TP, N], F32, kind="ExternalOutput")

    AF = mybir.ActivationFunctionType
    with tile.TileContext(nc) as tc:
        with (
            tc.tile_pool(name="const", bufs=1) as cpool,
            tc.tile_pool(name="io", bufs=3) as iopool,
            tc.tile_pool(name="work", bufs=4) as wpool,
            tc.tile_pool(name="state", bufs=3) as spool,
            tc.tile_pool(name="psA", bufs=1, space="PSUM") as psA,
            tc.tile_pool(name="psB", bufs=1, space="PSUM") as psB,
        ):
            # ---- constants ----
            at_sb = cpool.tile([128, MK, N], F32R, name="at_sb")   # A_in.T rows
            a_sb = cpool.tile([128, MK, N], F32R, name="a_sb")     # A_in rows
            for k in range(MK):
                nc.sync.dma_start(at_sb[:, k, :], ain_t_d.ap()[128 * k:128 * (k + 1), :].bitcast(F32R))
                nc.sync.dma_start(a_sb[:, k, :], ain_d.ap()[128 * k:128 * (k + 1), :].bitcast(F32R))
            w_sb = {}
            for w in wnames:
                w_sb[w] = cpool.tile([TP, TP], F32R, name=f"{w}_sb")
                nc.sync.dma_start(w_sb[w][:], w_d[w].ap().bitcast(F32R))
            bias_sb = cpool.tile([TP, 1], F32, name="bias_sb")
            nc.sync.dma_start(bias_sb[:], bias_d.ap())
            ident = cpool.tile([128, 128], F32R, name="ident")
            nc.sync.dma_start(ident[:], ident_d.ap().bitcast(F32R))

            # ---- per-tile pipeline, emitted as a 3-deep wavefront ----
            # Wave w emits (i=w, t=0), (i=w-1, t=1), (i=w-2, t=2) so every
            # engine's FIFO interleaves three independent tile chains.
            st = [dict() for _ in range(NT)]

            def emit_step(i, t):
                if t == 0:
                    xl1_sb = iopool.tile([128, MK, TP], F32R, name="xl1_sb")
                    nc.sync.dma_start(xl1_sb[:], xl1_d.ap()[i].bitcast(F32R))
                    xl2_sb = iopool.tile([TP, N], F32R, name="xl2_sb", bufs=4)
                    nc.sync.dma_start(xl2_sb[:], xl2_d.ap()[i].bitcast(F32R))
                    st[i]["xl1"] = xl1_sb
                    st[i]["xl2"] = xl2_sb
                    st[i]["fn"] = xl2_sb      # step-0 node state is x itself
                    # skip-connection projection of x is step-invariant
                    ox_ps = psB.tile([TP, N], F32, name="ox_ps")
                    nc.tensor.matmul(ox_ps[:], w_sb["wo_x"][:], xl2_sb[:],
                                     start=True, stop=True)
                    ox_sb = wpool.tile([TP, N], F32, name="ox_sb", bufs=4)
                    nc.vector.tensor_copy(ox_sb[:], ox_ps[:])
                    st[i]["ox"] = ox_sb
                xl1_sb = st[i]["xl1"]
                xl2_sb = st[i]["xl2"]
                fn_sb = st[i]["fn"]
                fnl1_sb = st[i].get("fnl1")
                ox_sb = st[i]["ox"]
                if True:
                    # aggregation: agg = nodes_L1.T @ A~  -> L2 layout
                    agg_in_ps = psA.tile([TP, N], F32, name="agg_in_ps")
                    agg_out_ps = psA.tile([TP, N], F32, name="agg_out_ps")
                    lhs = xl1_sb if t == 0 else fnl1_sb
                    for k in range(MK):
                        nc.tensor.matmul(agg_in_ps[:], lhs[:, k, :],
                                         at_sb[:, k, :],
                                         start=(k == 0), stop=(k == MK - 1))
                    for k in range(MK):
                        nc.tensor.matmul(agg_out_ps[:], lhs[:, k, :],
                                         a_sb[:, k, :],
                                         start=(k == 0), stop=(k == MK - 1))
                    agg_in_sb = wpool.tile([TP, N], F32R, name="agg_in_sb")
                    agg_out_sb = wpool.tile([TP, N], F32R, name="agg_out_sb")
                    nc.scalar.copy(agg_in_sb[:], agg_in_ps[:])
                    nc.scalar.copy(agg_out_sb[:], agg_out_ps[:])

                    # gates: z and r share one 2-bank psum tile -> one sigmoid
                    zr_ps = psB.tile([TP, 2, N], F32, name="zr_ps")
                    nc.tensor.matmul(zr_ps[:, 0, :], w_sb["wz_in"][:], agg_in_sb[:], start=True, stop=False)
                    nc.tensor.matmul(zr_ps[:, 0, :], w_sb["wz_out"][:], agg_out_sb[:], start=False, stop=False)
                    nc.tensor.matmul(zr_ps[:, 0, :], w_sb["wz_fn"][:], fn_sb[:], start=False, stop=True)
                    nc.tensor.matmul(zr_ps[:, 1, :], w_sb["wr_in"][:], agg_in_sb[:], start=True, stop=False)
                    nc.tensor.matmul(zr_ps[:, 1, :], w_sb["wr_out"][:], agg_out_sb[:], start=False, stop=False)
                    nc.tensor.matmul(zr_ps[:, 1, :], w_sb["wr_fn"][:], fn_sb[:], start=False, stop=True)
                    zr_sb = wpool.tile([TP, 2, N], F32, name="zr_sb")
                    nc.scalar.activation(zr_sb[:], zr_ps[:], AF.Sigmoid)
                    z_sb = zr_sb[:, 0, :]
                    r_sb = zr_sb[:, 1, :]
                    rf_sb = wpool.tile([TP, N], F32R, name="rf_sb")
                    nc.vector.tensor_mul(rf_sb[:], r_sb, fn_sb[:].bitcast(F32))
                    # zf1 = (z-1)*fn, off the tanh critical path (GpSimd)
                    zf1_sb = wpool.tile([TP, N], F32, name="zf1_sb")
                    nc.vector.scalar_tensor_tensor(
                        zf1_sb[:], z_sb, 1.0, fn_sb[:].bitcast(F32),
                        op0=mybir.AluOpType.subtract, op1=mybir.AluOpType.mult)

                    h_ps = psB.tile([TP, N], F32, name="h_ps")
                    nc.tensor.matmul(h_ps[:], w_sb["wh_in"][:], agg_in_sb[:], start=True, stop=False)
                    nc.tensor.matmul(h_ps[:], w_sb["wh_out"][:], agg_out_sb[:], start=False, stop=False)
                    nc.tensor.matmul(h_ps[:], w_sb["wh_fn"][:], rf_sb[:], start=False, stop=True)
                    h_sb = wpool.tile([TP, N], F32, name="h_sb")
                    nc.scalar.activation(h_sb[:], h_ps[:], AF.Tanh)

                    # fn' = fn + z*(h - fn) = z*h - (z-1)*fn
                    zh_sb = wpool.tile([TP, N], F32, name="zh_sb")
                    nc.vector.tensor_mul(zh_sb[:], z_sb, h_sb[:])
                    fnn_sb = spool.tile([TP, N], F32R, name="fnn_sb", bufs=4)
                    nc.vector.tensor_sub(fnn_sb[:], zh_sb[:], zf1_sb[:])

                    # output projection: o = wo_fn@fn' + (hoisted x part) + bias
                    o_ps = psB.tile([TP, N], F32, name="o_ps")
                    nc.tensor.matmul(o_ps[:], w_sb["wo_fn"][:], fnn_sb[:], start=True, stop=True)
                    o_sb = iopool.tile([TP, N], F32, name="o_sb")
                    nc.vector.scalar_tensor_tensor(
                        o_sb[:], o_ps[:], bias_sb[:], ox_sb[:],
                        op0=mybir.AluOpType.add, op1=mybir.AluOpType.add)
                    nc.sync.dma_start(out_d.ap()[t, i], o_sb[:])

                    # transpose fn' into L1 for the next step's aggregation
                    if t < T - 1:
                        tp_ps = psA.tile([128, MK, TP], F32R, name="tp_ps")
                        fnl1_sb = spool.tile([128, MK, TP], F32R, name="fnl1_sb", bufs=4)
                        for k in range(MK):
                            nc.tensor.transpose(
                                tp_ps[:, k, :],
                                fnn_sb[:, 128 * k:128 * (k + 1)],
                                ident[0:TP, 0:TP])
                        nc.scalar.copy(fnl1_sb[:], tp_ps[:])
                        st[i]["fnl1"] = fnl1_sb
                    st[i]["fn"] = fnn_sb

            for w in range(NT + T - 1):
                for t in range(T):
                    i = w - t
                    if 0 <= i < NT:
                        emit_step(i, t)

    nc.compile()
    return nc


_NC_CACHE = None


def _get_nc():
    global _NC_CACHE
    if _NC_CACHE is None:
        _NC_CACHE = build_nc()
    return _NC_CACHE


def _host_prep(x, A_in, W3w, W3u, W4w, W4u, W5w, W5u, W_out, b_out):
    f32 = np.float32
    eye = np.eye(BPER, dtype=f32)

    def blk(w):
        return np.ascontiguousarray(np.kron(eye, np.asarray(w, f32).T))

    shared = {
        "ain_t": np.ascontiguousarray(np.asarray(A_in, f32).T),
        "ain": np.ascontiguousarray(np.asarray(A_in, f32)),
        "wz_in": blk(W3w[:, :H]), "wz_out": blk(W3w[:, H:]), "wz_fn": blk(W3u),
        "wr_in": blk(W4w[:, :H]), "wr_out": blk(W4w[:, H:]), "wr_fn": blk(W4u),
        "wh_in": blk(W5w[:, :H]), "wh_out": blk(W5w[:, H:]), "wh_fn": blk(W5u),
        "wo_fn": blk(W_out[:, :H]), "wo_x": blk(W_out[:, H:]),
        "bias": np.ascontiguousarray(
            np.tile(np.asarray(b_out, f32), BPER)[:, None]),
        "ident": np.eye(128, dtype=f32),
    }

    in_maps = []
    x = np.asarray(x, f32)
    for c in range(NCORES):
        xp = np.zeros((BPAD, N, H), f32)
        xp[:BS] = x[BS * c:BS * (c + 1)]
        # L1: [m, (b,h)] -> dram [NT, 128(p), MK(k), TP(j)], m = 128k+p
        l1 = xp.transpose(1, 0, 2).reshape(N, NT, TP).transpose(1, 0, 2)
        l1 = l1.reshape(NT, MK, 128, TP).transpose(0, 2, 1, 3)
        # L2: [(b,h), n] -> dram [NT, TP, N]
        l2 = xp.transpose(0, 2, 1).reshape(NT, TP, N)
        in_maps.append({"xl1": np.ascontiguousarray(l1),
                        "xl2": np.ascontiguousarray(l2), **shared})
    return in_maps


def kernel(x, A_in, W3w, W3u, W4w, W4u, W5w, W5u, W_out, b_out):
    global LAST_RESULTS
    nc = _get_nc()
    in_maps = _host_prep(x, A_in, W3w, W3u, W4w, W4u, W5w, W5u, W_out, b_out)
    res = run_bass_kernel_spmd(nc, in_maps, core_ids=list(range(NCORES)))
    LAST_RESULTS = res
    outs = []
    for c in range(NCORES):
        o = res.results[c]["out"]                      # [T, NT, TP, N]
        o = o.reshape(T, NT, BPER, H, N).transpose(0, 1, 2, 4, 3)
        outs.append(o.reshape(T, BPAD, N, H)[:, :BS])  # drop pad batches
    return np.ascontiguousarray(np.concatenate(outs, axis=1))



# revision 26
# speedup vs baseline: 1.4969x; 1.4969x over previous
"""Trainium2 Bass kernel for the KGTM-style GRU message-passing GNN.

Reference math (per time step, T=3):
    agg_in  = A_in  @ nodes          (per batch)
    agg_out = A_in.T @ nodes
    zv = sigmoid(agg_in@Wz_in.T + agg_out@Wz_out.T + fn@W3u.T)
    rv = sigmoid(agg_in@Wr_in.T + agg_out@Wr_out.T + fn@W4u.T)
    hv = tanh   (agg_in@Wh_in.T + agg_out@Wh_out.T + (rv*fn)@W5u.T)
    fn' = fn + zv*(hv - fn)
    out_t = fn'@Wo_fn.T + x@Wo_x.T + b_out

Mapping: pure data parallel over batch (8 cores x 256 batches, padded to 258
= 43 tiles of 6).  On-chip layout "L2" puts (batch-local, channel) on the
128-partition axis (6*20 = 120 partitions) and node index n (512) on the
free axis.  Aggregation consumes nodes in layout "L1" [m, (b,h)] as the
matmul stationary operand so output lands directly in L2.

Speed strategy vs the fp32r baseline:
  * Aggregation runs as fp8(e4m3) DoubleRow matmuls: two 128-row
    contraction chunks per pass (K=256), A pre-scaled by 512 into fp8
    range, node state quantized to fp8 on the fly.  4 MMs instead of 8,
    each at half cost/row.
  * Gate "aggregation" contributions use DoubleRow with the two planes
    carrying (agg_in, agg_out): one MM per gate instead of two; the agg
    copy to SBUF writes an interleaved [TP, 2, N] fp8 operand.
  * Gate fn-side contributions, output projection and PE transposes run
    in bf16 (same PE rate as fp32r but enables fast DVE elementwise and
    halves state SBUF/DMA).
  * All elementwise state math is bf16 (DVE 2x mode); rf and (h-fn) are
    placed on GpSimd to balance engine load; the Wo_x@x + bias term is
    folded into the output matmul via an appended ones-row in xl2.
  * Output is stored/DMAed as bf16 and upcast on the host.
  * Tiles advance three at a time through the 3-step wavefront (9 chains
    in flight) so the long cross-engine dependency chain per step is
    hidden by engine throughput.
Scale bookkeeping: A*32 -> agg_psum = stk = 32*agg (fp8-friendly, plain
copy); fp8 gate weights *8 and bf16 fn weights *256 -> gate preacts 256x
true; sigmoid/tanh applied with scale 1/256.
"""

import numpy as np
import ml_dtypes

import concourse.bacc as bacc
import concourse.tile as tile
import concourse.mybir as mybir
from concourse.bass_utils import run_bass_kernel_spmd

F32 = mybir.dt.float32
BF16 = mybir.dt.bfloat16
FP8 = mybir.dt.float8e4
DR = mybir.MatmulPerfMode.DoubleRow

B, N, H, T = 2048, 512, 20, 3
NCORES = 8
BS = B // NCORES          # 256 batches per core
BPER = 6                  # batches per partition tile
TP = BPER * H             # 120 partitions per tile
NT = 43                   # tiles per core (43*6 = 258, 2 batches zero pad)
BPAD = NT * BPER          # 258

ASCALE = 32.0             # A scaled into fp8 range; agg psum = 32*agg_true
WAGG = 8.0                # fp8 gate-weight scale (8*32 = 256x at PSUM)
WFN = 256.0               # bf16 fn-side gate-weight scale
SACT = 1.0 / 256.0        # sigmoid/tanh input descale

NP_BF16 = ml_dtypes.bfloat16
NP_FP8 = ml_dtypes.float8_e4m3fn

LAST_RESULTS = None       # stash of the most recent BassKernelResults


def build_nc():
    nc = bacc.Bacc("TRN2", target_bir_lowering=False, debug=False,
                   num_devices=NCORES)

    xl1_d = nc.dram_tensor("xl1", [NT, 128, 4, 128], FP8, kind="ExternalInput")
    xl2_d = nc.dram_tensor("xl2", [NT, TP + 1, N], BF16, kind="ExternalInput")
    at8_d = nc.dram_tensor("at8", [128, 2, 2, N], FP8, kind="ExternalInput")
    a8_d = nc.dram_tensor("a8", [128, 2, 2, N], FP8, kind="ExternalInput")
    w8_d = {w: nc.dram_tensor(w, [TP, 2, 128], FP8, kind="ExternalInput")
            for w in ("wz8", "wr8", "wh8")}
    wb_d = {w: nc.dram_tensor(w, [TP, TP], BF16, kind="ExternalInput")
            for w in ("wzfn", "wrfn", "whfn", "wofn")}
    wox_d = nc.dram_tensor("wox", [TP + 1, TP], BF16, kind="ExternalInput")
    identb_d = nc.dram_tensor("identb", [128, 128], BF16, kind="ExternalInput")
    out_d = nc.dram_tensor("out", [T, NT, TP, N], BF16, kind="ExternalOutput")

    AF = mybir.ActivationFunctionType
    with tile.TileContext(nc) as tc:
        with (
            tc.tile_pool(name="const", bufs=1) as cpool,
            tc.tile_pool(name="io", bufs=9) as iopool,
            tc.tile_pool(name="work", bufs=12) as wpool,
            tc.tile_pool(name="state", bufs=12) as spool,
            tc.tile_pool(name="psA", bufs=1, space="PSUM") as psA,
            tc.tile_pool(name="psB", bufs=1, space="PSUM") as psB,
        ):
            # ---- constants ----
            at8 = cpool.tile([128, 2, 2, N], FP8, name="at8")
            nc.sync.dma_start(at8[:], at8_d.ap())
            a8 = cpool.tile([128, 2, 2, N], FP8, name="a8")
            nc.sync.dma_start(a8[:], a8_d.ap())
            w8 = {}
            for w in ("wz8", "wr8", "wh8"):
                w8[w] = cpool.tile([TP, 2, 128], FP8, name=f"{w}_sb")
                nc.sync.dma_start(w8[w][:], w8_d[w].ap())
            wb = {}
            for w in ("wzfn", "wrfn", "whfn", "wofn"):
                wb[w] = cpool.tile([TP, TP], BF16, name=f"{w}_sb")
                nc.sync.dma_start(wb[w][:], wb_d[w].ap())
            wox = cpool.tile([TP + 1, TP], BF16, name="wox_sb")
            nc.sync.dma_start(wox[:], wox_d.ap())
            identb = cpool.tile([128, 128], BF16, name="identb")
            nc.sync.dma_start(identb[:], identb_d.ap())

            # ---- per-tile pipeline, emitted as a 3-deep wavefront ----
            st = [dict() for _ in range(NT)]

            def emit_step(i, t):
                if t == 0:
                    xl1t = iopool.tile([128, 4, 128], FP8, name="xl1t")
                    nc.sync.dma_start(xl1t[:], xl1_d.ap()[i])
                    xl2t = iopool.tile([TP + 1, N], BF16, name="xl2t", bufs=12)
                    nc.sync.dma_start(xl2t[:], xl2_d.ap()[i])
                    st[i]["xl2"] = xl2t
                    st[i]["fn"] = xl2t[0:TP, :]
                    st[i]["fnl1"] = xl1t
                xl2t = st[i]["xl2"]
                fn_ap = st[i]["fn"]
                fnl1 = st[i]["fnl1"]

                # aggregation: fp8 DoubleRow, K=256 per MM (chunk pairs)
                agg_in_ps = psA.tile([TP, N], F32, name="agg_in_ps")
                agg_out_ps = psA.tile([TP, N], F32, name="agg_out_ps")
                for pr in range(2):
                    nc.tensor.matmul(agg_in_ps[:],
                                     fnl1[:, 2 * pr:2 * pr + 2, 0:TP],
                                     at8[:, pr, :, :],
                                     start=(pr == 0), stop=(pr == 1),
                                     perf_mode=DR)
                for pr in range(2):
                    nc.tensor.matmul(agg_out_ps[:],
                                     fnl1[:, 2 * pr:2 * pr + 2, 0:TP],
                                     a8[:, pr, :, :],
                                     start=(pr == 0), stop=(pr == 1),
                                     perf_mode=DR)

                # agg -> interleaved fp8 moving operand (planes: in, out)
                stk = wpool.tile([TP, 2, N], FP8, name="stk")
                nc.vector.tensor_copy(stk[:, 0, :], agg_in_ps[:])
                nc.vector.tensor_copy(stk[:, 1, :], agg_out_ps[:])

                # gates: z,r share a 2-bank psum -> one sigmoid
                zr_ps = psB.tile([TP, 2, N], F32, name="zr_ps")
                nc.tensor.matmul(zr_ps[:, 0, :], w8["wz8"][:, :, 0:TP], stk[:],
                                 start=True, stop=False, perf_mode=DR)
                nc.tensor.matmul(zr_ps[:, 0, :], wb["wzfn"][:], fn_ap,
                                 start=False, stop=True)
                nc.tensor.matmul(zr_ps[:, 1, :], w8["wr8"][:, :, 0:TP], stk[:],
                                 start=True, stop=False, perf_mode=DR)
                nc.tensor.matmul(zr_ps[:, 1, :], wb["wrfn"][:], fn_ap,
                                 start=False, stop=True)
                zr_sb = wpool.tile([TP, 2, N], BF16, name="zr_sb")
                nc.scalar.activation(zr_sb[:], zr_ps[:], AF.Sigmoid, scale=SACT)

                rf = wpool.tile([TP, N], BF16, name="rf")
                nc.gpsimd.tensor_mul(rf[:], zr_sb[:, 1, :], fn_ap)

                h_ps = psB.tile([TP, N], F32, name="h_ps")
                nc.tensor.matmul(h_ps[:], w8["wh8"][:, :, 0:TP], stk[:],
                                 start=True, stop=False, perf_mode=DR)
                nc.tensor.matmul(h_ps[:], wb["whfn"][:], rf[:],
                                 start=False, stop=True)
                h_sb = wpool.tile([TP, N], BF16, name="h_sb")
                nc.scalar.activation(h_sb[:], h_ps[:], AF.Tanh, scale=SACT)

                # fn' = fn + z*(h - fn)
                dt_ = wpool.tile([TP, N], BF16, name="dt")
                nc.gpsimd.tensor_sub(dt_[:], h_sb[:], fn_ap)
                et = wpool.tile([TP, N], BF16, name="et")
                nc.vector.tensor_mul(et[:], zr_sb[:, 0, :], dt_[:])
                fnn = spool.tile([TP, N], BF16, name="fnn")
                nc.vector.tensor_add(fnn[:], fn_ap, et[:])

                # out = Wo_fn@fn' + Wo_x@x + bias (ones-row in xl2)
                o_ps = psB.tile([TP, N], F32, name="o_ps")
                nc.tensor.matmul(o_ps[:], wb["wofn"][:], fnn[:],
                                 start=True, stop=False)
                nc.tensor.matmul(o_ps[:], wox[:], xl2t[:],
                                 start=False, stop=True)
                o_sb = iopool.tile([TP, N], BF16, name="o_sb")
                nc.scalar.copy(o_sb[:], o_ps[:])
                nc.sync.dma_start(out_d.ap()[t, i], o_sb[:])

                # transpose fn' into fp8 L1 for next step's aggregation
                if t < T - 1:
                    tp_ps = psA.tile([128, 4, 128], BF16, name="tp_ps")
                    for k in range(4):
                        nc.tensor.transpose(
                            tp_ps[:, k, 0:TP],
                            fnn[:, 128 * k:128 * (k + 1)],
                            identb[0:TP, 0:TP])
                    fnl1n = spool.tile([128, 4, 128], FP8, name="fnl1n")
                    nc.vector.tensor_copy(fnl1n[:, :, 0:TP], tp_ps[:, :, 0:TP])
                    st[i]["fnl1"] = fnl1n
                st[i]["fn"] = fnn[:]

            # Tiles advance in pairs: 6 independent chains in flight, so the
            # long per-step dependency chain is hidden by engine throughput.
            npair = (NT + 2) // 3
            for w in range(npair + T - 1):
                for t in range(T):
                    for j in range(3):
                        i = 3 * (w - t) + j
                        if 0 <= i < NT:
                            emit_step(i, t)

    nc.compile()
    return nc


_NC_CACHE = None


def _get_nc():
    global _NC_CACHE
    if _NC_CACHE is None:
        _NC_CACHE = build_nc()
    return _NC_CACHE


def _to_fp8(a):
    return np.clip(np.asarray(a, np.float32), -240.0, 240.0).astype(NP_FP8)


def _to_bf16(a):
    return np.asarray(a, np.float32).astype(NP_BF16)


def _pair_layout(m):
    """[512, N] (contraction-major) -> [128, pair2, plane2, N]."""
    return np.ascontiguousarray(
        m.reshape(2, 2, 128, N).transpose(2, 0, 1, 3))


def _host_prep(x, A_in, W3w, W3u, W4w, W4u, W5w, W5u, W_out, b_out):
    f32 = np.float32
    eye = np.eye(BPER, dtype=f32)

    def blk(w, scale):
        return np.kron(eye, np.asarray(w, f32).T) * scale

    A = np.asarray(A_in, f32)
    w8 = {}
    for name, w in (("wz8", W3w), ("wr8", W4w), ("wh8", W5w)):
        arr = np.zeros((TP, 2, 128), f32)
        arr[:, 0, :TP] = blk(np.asarray(w, f32)[:, :H], WAGG)
        arr[:, 1, :TP] = blk(np.asarray(w, f32)[:, H:], WAGG)
        w8[name] = _to_fp8(arr)
    wox = np.zeros((TP + 1, TP), f32)
    wox[:TP] = blk(np.asarray(W_out, f32)[:, H:], 1.0)
    wox[TP] = np.tile(np.asarray(b_out, f32), BPER)

    shared = {
        "at8": _to_fp8(_pair_layout(A.T * ASCALE)),
        "a8": _to_fp8(_pair_layout(A * ASCALE)),
        **w8,
        "wzfn": _to_bf16(blk(W3u, WFN)),
        "wrfn": _to_bf16(blk(W4u, WFN)),
        "whfn": _to_bf16(blk(W5u, WFN)),
        "wofn": _to_bf16(blk(np.asarray(W_out, f32)[:, :H], 1.0)),
        "wox": _to_bf16(wox),
        "identb": _to_bf16(np.eye(128, dtype=f32)),
    }

    in_maps = []
    x = np.asarray(x, f32)
    for c in range(NCORES):
        xp = np.zeros((BPAD, N, H), f32)
        xp[:BS] = x[BS * c:BS * (c + 1)]
        # L1 fp8: [m, (b,h)] -> [NT, 128(p), 4(pair,plane), 128(pad)]
        l1 = xp.transpose(1, 0, 2).reshape(N, NT, TP)
        l1 = l1.reshape(2, 2, 128, NT, TP).transpose(3, 2, 0, 1, 4)
        xl1 = np.zeros((NT, 128, 4, 128), f32)
        xl1[:, :, :, :TP] = l1.reshape(NT, 128, 4, TP)
        # L2 bf16: [(b,h), n] + ones row for the bias fold
        xl2 = np.ones((NT, TP + 1, N), f32)
        xl2[:, :TP, :] = xp.transpose(0, 2, 1).reshape(NT, TP, N)
        in_maps.append({"xl1": _to_fp8(xl1), "xl2": _to_bf16(xl2), **shared})
    return in_maps


def kernel(x, A_in, W3w, W3u, W4w, W4u, W5w, W5u, W_out, b_out):
    global LAST_RESULTS
    nc = _get_nc()
    in_maps = _host_prep(x, A_in, W3w, W3u, W4w, W4u, W5w, W5u, W_out, b_out)
    res = run_bass_kernel_spmd(nc, in_maps, core_ids=list(range(NCORES)))
    LAST_RESULTS = res
    outs = []
    for c in range(NCORES):
        o = np.asarray(res.results[c]["out"]).astype(np.float32)
        o = o.reshape(T, NT, BPER, H, N).transpose(0, 1, 2, 4, 3)
        outs.append(o.reshape(T, BPAD, N, H)[:, :BS])  # drop pad batches
    return np.ascontiguousarray(np.concatenate(outs, axis=1))
